# revision 55
# baseline (speedup 1.0000x reference)
"""DifferentiableLengthRegulator Trainium2 kernel.

out[b,c,l] = y_mask * (sum_t x[b,c,t]*W[b,t,l]) / (sum_t W[b,t,l] + eps)
W = exp(-0.5*(l - center[b,t])^2 / (w[b,t]^2*sigma_scale^2 + eps))

Sharding: data-parallel over batch B=16 -> 8 cores x 2 batches.
Per core, per batch (banded over the frame axis, since the Gaussian
weights vanish outside ~13 sigma of each token chunk's centers):
  DVE : mu = pos - c                        (tensor_scalar, 2x fp32)
  ACT : W  = DerivErf(s*mu) -> bf16         (= 2/sqrt(pi) * exp(-(s*mu)^2);
        the 2/sqrt(pi) factor cancels in the normalization)
  PE  : psum[l,0:257] = sum_tc W_tc[:,l-slice]^T @ [xT | ones]  (bf16)
  DVE/ACT/POOL: rd = y_mask/(psum[:,256]+eps);
        out_sb[l,c] = psum[l,0:256]*rd (PSUM->SBUF move, engine-balanced)
Output written (B, L, C)-contiguous; host returns the transpose view.
"""

import numpy as np
import ml_dtypes

B, C, T, L = 16, 256, 512, 4096
N_CORES = 8
BPC = B // N_CORES  # batches per core
CH = 128            # partition chunk
TCN = T // CH       # 4 token chunks
LCN = L // CH       # 32 frame chunks
GRP = 2             # frame chunks per psum group
NGRP = LCN // GRP   # 16 groups
EPS = 1e-8
MARGIN_SIGMA = 13.19
BAND_ALIGN = 128

_bf16 = ml_dtypes.bfloat16
_cache = {}


def _center_scale(w, sigma_scale):
    """Mirror the reference's cumsum/center math (same jax backend bits)."""
    try:
        import jax.numpy as jnp

        wj = jnp.asarray(w)
        center = np.asarray(jnp.cumsum(wj, axis=1) - 0.5 * wj, dtype=np.float32)
    except Exception:
        center = (np.cumsum(w, axis=1, dtype=np.float32) - 0.5 * w).astype(np.float32)
    sigma = (w * np.float32(sigma_scale)).astype(np.float32)
    # W = DerivErf(s*mu)*sqrt(pi)/2 = exp(-(s*mu)^2), s = sqrt(0.5/(sig^2+eps))
    s = np.sqrt(np.float32(0.5) / (np.square(sigma) + np.float32(EPS))).astype(np.float32)
    return center, s


def _bands(center, w_all):
    """Per (slot, tc) aligned frame band, unioned across cores (SPMD)."""
    margin = float(MARGIN_SIGMA * w_all.max() + 1.0)
    bands = []
    for slot in range(BPC):
        rows = center[slot::BPC]  # the 8 batches that land in this slot
        sb = []
        for tc in range(TCN):
            seg = rows[:, tc * CH:(tc + 1) * CH]
            bs = max(0, int(np.floor((seg.min() - margin) / BAND_ALIGN)) * BAND_ALIGN)
            be = min(L, int(np.ceil((seg.max() + margin) / BAND_ALIGN)) * BAND_ALIGN)
            if tc == 0:
                bs = 0
            if tc == TCN - 1:
                be = L
            bs = min(bs, be - CH)
            sb.append((bs, be))
        bands.append(sb)
    return bands


def _split_excess_waits(nc, max_waits=1):
    """walrus here caps sync-waits at 1 per compute instruction; move the
    excess onto injected same-engine NoOps just before the instruction
    (waiting earlier on the same engine is always safe)."""
    from concourse import mybir

    for f in nc.m.functions:
        for blk in f.blocks:
            new = []
            for inst in blk.instructions:
                si = inst.sync_info
                if si is not None and len(si.on_wait) > max_waits:
                    waits = list(si.on_wait)
                    keep, extra = waits[-max_waits:], waits[:-max_waits]
                    for i in range(0, len(extra), max_waits):
                        nop = mybir.InstNoOp(name=f"{inst.name}-xw{i}", ins=[], outs=[])
                        nop.engine = inst.engine
                        nop.sync_info = mybir.SyncInfo(
                            on_wait=extra[i:i + max_waits], on_update=[])
                        new.append(nop)
                    inst.sync_info = mybir.SyncInfo(
                        on_wait=keep, on_update=list(si.on_update))
                new.append(inst)
            blk.instructions = new


def _build(band_key):
    import concourse.bass as bass
    import concourse.tile as tile
    from concourse import mybir

    band_key, trivial_masks = band_key
    bands = [[(band_key[s][t][0], band_key[s][t][1]) for t in range(TCN)]
             for s in range(BPC)]
    wmax = max(be - bs for sb in bands for (bs, be) in sb)

    nc = bass.Bass("TRN2", target_bir_lowering=False, debug=False)
    # xta host layout: [b, p, tc, c] so the DMA is descriptor-light
    xta_d = nc.declare_dram_parameter("xta", [BPC, CH, TCN, C + 1], mybir.dt.bfloat16, isOutput=False)
    coefs_d = nc.declare_dram_parameter("coefs", [CH, 3 * BPC * TCN], mybir.dt.float32, isOutput=False)
    ym_d = nc.declare_dram_parameter("ym", [CH, BPC * LCN], mybir.dt.float32, isOutput=False)
    out_d = nc.declare_dram_parameter("out", [BPC, L, C], mybir.dt.float32, isOutput=True)

    f32 = mybir.dt.float32
    bf16 = mybir.dt.bfloat16
    FT = mybir.ActivationFunctionType
    OP = mybir.AluOpType

    def bcast(ap_col, n):
        return bass.AP(tensor=ap_col.tensor, offset=ap_col.offset,
                       ap=list(ap_col.ap) + [[0, n]])

    with tile.TileContext(nc) as tc_:
        import contextlib

        with contextlib.ExitStack() as ctx:
            consts = ctx.enter_context(tc_.tile_pool(name="consts", bufs=1))
            xta_p = ctx.enter_context(tc_.tile_pool(name="xta", bufs=2))
            mu_p = ctx.enter_context(tc_.tile_pool(name="mu", bufs=3))
            w_pools = [ctx.enter_context(tc_.tile_pool(name=f"w{t}", bufs=2)) for t in range(TCN)]
            psum_p = ctx.enter_context(tc_.tile_pool(name="ps", bufs=4, space="PSUM"))
            small_p = ctx.enter_context(tc_.tile_pool(name="small", bufs=10))
            out_p = ctx.enter_context(tc_.tile_pool(name="osb", bufs=4))

            # --- constants. pos comes entirely from iota on GpSimd (no DMA
            # wait at all); coefs on the sync HWDGE ring (keep ScalarE free
            # for the ACT table load -- dma_start blocks its issuing engine).
            coefs_sb = consts.tile([CH, 3 * BPC * TCN], f32)
            nc.sync.dma_start(out=coefs_sb, in_=coefs_d[:, :])
            pos_f = consts.tile([CH, L], f32)
            IW = max(bands[s][0][1] for s in range(BPC))
            IW2 = max(bands[s][1][1] for s in range(BPC))
            for lo, hi in ((0, IW), (IW, IW2), (IW2, L)):
                nc.gpsimd.iota(pos_f[:, lo:hi], pattern=[[1, hi - lo]], base=lo,
                               channel_multiplier=0,
                               allow_small_or_imprecise_dtypes=True)
            ym_sb = consts.tile([CH, BPC * LCN], f32)
            # W carries DerivErf's 2/sqrt(pi) factor; scaling eps by the same
            # factor makes rd = ym/(k*sumW + k*eps) = ym/k/(sumW + eps) exact.
            eps_sb = consts.tile([CH, 1], f32)
            nc.vector.memset(eps_sb, float(EPS) * 2.0 / np.pi ** 0.5)
            # warm the ACT spline tables during the input DMAs
            tblw = consts.tile([CH, 1], f32)
            nc.scalar.activation(out=tblw, in_=eps_sb, func=FT.Derivative_Erf)

            def col(tile_, idx):
                return tile_[:, idx:idx + 1]

            def cidx(q, b, t):
                return (q * BPC + b) * TCN + t

            xta_tiles = {}
            w_tiles = {}

            def load_xta(b):
                xta_sb = xta_p.tile([CH, TCN, C + 1], bf16)
                nc.sync.dma_start(out=xta_sb, in_=xta_d[b])
                if not trivial_masks:
                    for t in range(TCN):
                        # x_mask fold on GpSimd (broadcast mult, x cols only)
                        nc.gpsimd.tensor_tensor(
                            out=xta_sb[:, t, :C], in0=xta_sb[:, t, :C],
                            in1=bcast(col(coefs_sb, cidx(2, b, t)), C),
                            op=OP.mult,
                        )
                xta_tiles[b] = xta_sb

            def wgen(b, t):
                bs, be = bands[b][t]
                bw = be - bs
                # The first two mu tiles run on DVE at 2x while GpSimd is
                # still busy with iota; later ones go to GpSimd so DVE can
                # focus on the PSUM-recycling normalize chain.
                mu = mu_p.tile([CH, wmax], f32, tag="mu")
                if b == 0 and t < 2:
                    nc.vector.tensor_scalar(
                        out=mu[:, :bw], in0=pos_f[:, bs:be],
                        scalar1=col(coefs_sb, cidx(0, b, t)), scalar2=None,
                        op0=OP.subtract,
                    )
                else:
                    nc.gpsimd.tensor_tensor(
                        out=mu[:, :bw], in0=pos_f[:, bs:be],
                        in1=bcast(col(coefs_sb, cidx(0, b, t)), bw),
                        op=OP.subtract,
                    )
                wt = w_pools[t].tile([CH, wmax], bf16)
                # W = 2/sqrt(pi) * exp(-(s*mu)^2); constant cancels via rd
                nc.scalar.activation(
                    out=wt[:, :bw], in_=mu[:, :bw], func=FT.Derivative_Erf,
                    scale=col(coefs_sb, cidx(1, b, t)),
                )
                w_tiles[(b, t)] = wt

            ogrp_live = {}

            def group(b, g):
                sb = bands[b]
                pgrp = psum_p.tile([CH, GRP, 512], f32, tag="pgrp")
                for k in range(GRP):
                    j = g * GRP + k
                    lo = j * CH
                    ctc = [t for t in range(TCN) if sb[t][0] <= lo and lo + CH <= sb[t][1]]
                    if not ctc:
                        nc.vector.memset(pgrp[:, k, :C + 1], 0.0)
                        continue
                    for i, t in enumerate(ctc):
                        off = lo - sb[t][0]
                        nc.tensor.matmul(
                            out=pgrp[:, k, :C + 1],
                            lhsT=w_tiles[(b, t)][:, off:off + CH],
                            rhs=xta_tiles[b][:, t, :],
                            start=(i == 0), stop=(i == len(ctc) - 1),
                        )
                dtmp = small_p.tile([CH, GRP], f32, tag="dtmp")
                # d + eps on DVE (same-engine chain: no cross-engine hop)
                nc.vector.tensor_scalar(
                    out=dtmp, in0=pgrp[:, :, C], scalar1=float(EPS) * 1.1283791670955126,
                    scalar2=None, op0=OP.add,
                )
                rd = small_p.tile([CH, GRP], f32, tag="rd")
                nc.vector.reciprocal(out=rd, in_=dtmp)
                if not trivial_masks:
                    nc.gpsimd.tensor_tensor(
                        out=rd, in0=rd,
                        in1=ym_sb[:, b * LCN + g * GRP: b * LCN + g * GRP + GRP],
                        op=OP.mult,
                    )
                # ogrp spans a PAIR of psum groups -> one out-DMA per pair
                half = g % 2
                if half == 0:
                    ogrp_new = out_p.tile([CH, 2 * GRP, C], f32, tag="ogrp")
                    ogrp_live[b] = ogrp_new
                ogrp = ogrp_live[b]
                osl = ogrp[:, half * GRP:(half + 1) * GRP, :]
                if (b * NGRP + g) % 3 < 2:
                    # normalize all chunks in one DVE op (rd broadcast on a
                    # stride-0 free dim)
                    rdb = bass.AP(tensor=rd.tensor, offset=rd.offset,
                                  ap=[rd.ap[0], rd.ap[1], [0, C]])
                    nc.vector.tensor_tensor(
                        out=osl, in0=pgrp[:, :, :C], in1=rdb, op=OP.mult,
                    )
                else:
                    for k in range(GRP):
                        nc.scalar.activation(
                            out=osl[:, k, :], in_=pgrp[:, k, :C],
                            func=FT.Copy, scale=col(rd, k),
                        )
                if half == 1:
                    pair = g // 2
                    eng = nc.sync if pair % 2 == 0 else nc.gpsimd
                    eng.dma_start(
                        out=out_d[b, pair * 2 * GRP * CH:(pair + 1) * 2 * GRP * CH, :]
                        .rearrange("(k p) c -> p k c", p=CH),
                        in_=ogrp,
                    )

            # batch 0 weight phase, then interleave batch 1's weight phase
            # into batch 0's matmul/normalize groups to keep all engines fed.
            # Emission order matters: each wgen's DMA-lane wait covers every
            # DMA emitted before it, so DMAs are interleaved to need-time.
            wgen(0, 0)          # needs iota1 + coefs only
            load_xta(0)
            wgen(0, 1)
            # PE warm-up on real data: back-to-back matmuls bridge the gap
            # until the group stream starts, so HAM un-throttles and the
            # real matmuls run at 2.4GHz.
            wps = psum_p.tile([CH, GRP, 512], f32, tag="pgrp")
            for i in range(12):
                nc.tensor.matmul(
                    out=wps[:, 0, :C + 1], lhsT=w_tiles[(0, 0)][:, :CH],
                    rhs=xta_tiles[0][:, 0, :], start=True, stop=True,
                )
            if not trivial_masks:
                nc.sync.dma_start(out=ym_sb, in_=ym_d[:, :])
            for t in range(2, TCN):
                wgen(0, t)
            load_xta(1)
            for g in range(NGRP):
                group(0, g)
                if g in (1, 3, 5, 7):
                    wgen(1, g // 2)
            for g in range(NGRP):
                group(1, g)
    return nc


def _prepare_inputs(x, w, x_mask, y_mask, sigma_scale):
    center, s = _center_scale(w, sigma_scale[0])
    bands = _bands(center, w)

    xt = np.ascontiguousarray(x.transpose(0, 2, 1))          # (B, T, C)
    xta = np.concatenate([xt, np.ones((B, T, 1), np.float32)], axis=2)
    # device layout [b, p, tc, c] for a descriptor-light DMA
    xta = np.ascontiguousarray(
        xta.reshape(B, TCN, CH, C + 1).transpose(0, 2, 1, 3)).astype(_bf16)

    xm = np.broadcast_to(x_mask.reshape(B, T), (B, T)).astype(np.float32)
    ymf = np.broadcast_to(y_mask.reshape(B, L), (B, L)).astype(np.float32)
    trivial_masks = bool(np.all(xm == 1.0) and np.all(ymf == 1.0))

    in_maps = []
    for core in range(N_CORES):
        bsel = [core * BPC + s_ for s_ in range(BPC)]
        coefs = np.empty((3, BPC, TCN, CH), np.float32)
        for s_, bb in enumerate(bsel):
            coefs[0, s_] = center[bb].reshape(TCN, CH)
            coefs[1, s_] = s[bb].reshape(TCN, CH)
            coefs[2, s_] = xm[bb].reshape(TCN, CH)
        ym_c = np.stack([ymf[bb].reshape(LCN, CH) for bb in bsel])  # (BPC,LCN,CH)
        in_maps.append({
            "xta": xta[bsel],
            "coefs": np.ascontiguousarray(
                coefs.reshape(3 * BPC * TCN, CH).T),          # [CH, 24]
            "ym": np.ascontiguousarray(
                ym_c.reshape(BPC * LCN, CH).T),               # [CH, 64]
        })
    band_key = (tuple(tuple(tuple(p) for p in sb) for sb in bands),
                trivial_masks)
    return in_maps, band_key


def kernel(x, w, x_mask, y_mask, sigma_scale):
    x = np.asarray(x, dtype=np.float32)
    w = np.asarray(w, dtype=np.float32)
    x_mask = np.asarray(x_mask, dtype=np.float32)
    y_mask = np.asarray(y_mask, dtype=np.float32)
    sigma_scale = np.asarray(sigma_scale, dtype=np.float32)
    assert x.shape == (B, C, T) and w.shape == (B, T)

    in_maps, band_key = _prepare_inputs(x, w, x_mask, y_mask, sigma_scale)

    if band_key not in _cache:
        nc = _build(band_key)
        _split_excess_waits(nc)
        _cache[band_key] = nc
    nc = _cache[band_key]

    from concourse.bass_utils import run_bass_kernel_spmd

    res = run_bass_kernel_spmd(nc, in_maps, list(range(N_CORES)), trace=False)
    outs = [res.results[i]["out"] for i in range(N_CORES)]      # (BPC, L, C) each
    full = np.concatenate(outs, axis=0)                          # (B, L, C)
    return full.transpose(0, 2, 1)                               # (B, C, L)


# revision 56
# speedup vs baseline: 1.0459x; 1.0459x over previous
"""DifferentiableLengthRegulator Trainium2 kernel.

out[b,c,l] = y_mask * (sum_t x[b,c,t]*W[b,t,l]) / (sum_t W[b,t,l] + eps)
W = exp(-0.5*(l - center[b,t])^2 / (w[b,t]^2*sigma_scale^2 + eps))

Sharding: data-parallel over batch B=16 -> 8 cores x 2 batches.
Per core, per batch (banded over the frame axis, since the Gaussian
weights vanish outside ~13 sigma of each token chunk's centers):
  DVE : mu = pos - c                        (tensor_scalar, 2x fp32)
  ACT : W  = DerivErf(s*mu) -> bf16         (= 2/sqrt(pi) * exp(-(s*mu)^2);
        the 2/sqrt(pi) factor cancels in the normalization)
  PE  : psum[l,0:257] = sum_tc W_tc[:,l-slice]^T @ [xT | ones]  (bf16)
  DVE/ACT/POOL: rd = y_mask/(psum[:,256]+eps);
        out_sb[l,c] = psum[l,0:256]*rd (PSUM->SBUF move, engine-balanced)
Output written (B, L, C)-contiguous; host returns the transpose view.
"""

import numpy as np
import ml_dtypes

B, C, T, L = 16, 256, 512, 4096
N_CORES = 8
BPC = B // N_CORES  # batches per core
CH = 128            # partition chunk
TCN = T // CH       # 4 token chunks
LCN = L // CH       # 32 frame chunks
GRP = 2             # frame chunks per psum group
NGRP = LCN // GRP   # 16 groups
EPS = 1e-8
MARGIN_SIGMA = 13.19
BAND_ALIGN = 128

_bf16 = ml_dtypes.bfloat16
_cache = {}


def _center_scale(w, sigma_scale):
    """Mirror the reference's cumsum/center math (same jax backend bits)."""
    try:
        import jax.numpy as jnp

        wj = jnp.asarray(w)
        center = np.asarray(jnp.cumsum(wj, axis=1) - 0.5 * wj, dtype=np.float32)
    except Exception:
        center = (np.cumsum(w, axis=1, dtype=np.float32) - 0.5 * w).astype(np.float32)
    sigma = (w * np.float32(sigma_scale)).astype(np.float32)
    # W = DerivErf(s*mu)*sqrt(pi)/2 = exp(-(s*mu)^2), s = sqrt(0.5/(sig^2+eps))
    s = np.sqrt(np.float32(0.5) / (np.square(sigma) + np.float32(EPS))).astype(np.float32)
    return center, s


def _bands(center, w_all):
    """Per (slot, tc) aligned frame band, unioned across cores (SPMD)."""
    margin = float(MARGIN_SIGMA * w_all.max() + 1.0)
    bands = []
    for slot in range(BPC):
        rows = center[slot::BPC]  # the 8 batches that land in this slot
        sb = []
        for tc in range(TCN):
            seg = rows[:, tc * CH:(tc + 1) * CH]
            bs = max(0, int(np.floor((seg.min() - margin) / BAND_ALIGN)) * BAND_ALIGN)
            be = min(L, int(np.ceil((seg.max() + margin) / BAND_ALIGN)) * BAND_ALIGN)
            if tc == 0:
                bs = 0
            if tc == TCN - 1:
                be = L
            bs = min(bs, be - CH)
            sb.append((bs, be))
        bands.append(sb)
    return bands


def _split_excess_waits(nc, max_waits=1):
    """walrus here caps sync-waits at 1 per compute instruction; move the
    excess onto injected same-engine NoOps just before the instruction
    (waiting earlier on the same engine is always safe)."""
    from concourse import mybir

    for f in nc.m.functions:
        for blk in f.blocks:
            new = []
            for inst in blk.instructions:
                si = inst.sync_info
                if si is not None and len(si.on_wait) > max_waits:
                    waits = list(si.on_wait)
                    keep, extra = waits[-max_waits:], waits[:-max_waits]
                    for i in range(0, len(extra), max_waits):
                        nop = mybir.InstNoOp(name=f"{inst.name}-xw{i}", ins=[], outs=[])
                        nop.engine = inst.engine
                        nop.sync_info = mybir.SyncInfo(
                            on_wait=extra[i:i + max_waits], on_update=[])
                        new.append(nop)
                    inst.sync_info = mybir.SyncInfo(
                        on_wait=keep, on_update=list(si.on_update))
                new.append(inst)
            blk.instructions = new


def _build(band_key):
    import concourse.bass as bass
    import concourse.tile as tile
    from concourse import mybir

    band_key, trivial_masks = band_key
    bands = [[(band_key[s][t][0], band_key[s][t][1]) for t in range(TCN)]
             for s in range(BPC)]
    wmax = max(be - bs for sb in bands for (bs, be) in sb)

    nc = bass.Bass("TRN2", target_bir_lowering=False, debug=False)
    # xta host layout: [b, p, tc, c] so the DMA is descriptor-light
    xta_d = nc.declare_dram_parameter("xta", [BPC, CH, TCN, C + 1], mybir.dt.bfloat16, isOutput=False)
    coefs_d = nc.declare_dram_parameter("coefs", [CH, 3 * BPC * TCN], mybir.dt.float32, isOutput=False)
    ym_d = nc.declare_dram_parameter("ym", [CH, BPC * LCN], mybir.dt.float32, isOutput=False)
    out_d = nc.declare_dram_parameter("out", [BPC, L, C], mybir.dt.float32, isOutput=True)

    f32 = mybir.dt.float32
    bf16 = mybir.dt.bfloat16
    FT = mybir.ActivationFunctionType
    OP = mybir.AluOpType

    def bcast(ap_col, n):
        return bass.AP(tensor=ap_col.tensor, offset=ap_col.offset,
                       ap=list(ap_col.ap) + [[0, n]])

    with tile.TileContext(nc) as tc_:
        import contextlib

        with contextlib.ExitStack() as ctx:
            consts = ctx.enter_context(tc_.tile_pool(name="consts", bufs=1))
            xta_p = ctx.enter_context(tc_.tile_pool(name="xta", bufs=2))
            mu_p = ctx.enter_context(tc_.tile_pool(name="mu", bufs=3))
            w_pools = [ctx.enter_context(tc_.tile_pool(name=f"w{t}", bufs=2)) for t in range(TCN)]
            psum_p = ctx.enter_context(tc_.tile_pool(name="ps", bufs=4, space="PSUM"))
            small_p = ctx.enter_context(tc_.tile_pool(name="small", bufs=6))
            out_p = ctx.enter_context(tc_.tile_pool(name="osb", bufs=4))

            # --- constants. pos comes entirely from iota on GpSimd (no DMA
            # wait at all); coefs on the sync HWDGE ring (keep ScalarE free
            # for the ACT table load -- dma_start blocks its issuing engine).
            coefs_sb = consts.tile([CH, 3 * BPC * TCN], f32)
            nc.sync.dma_start(out=coefs_sb, in_=coefs_d[:, :])
            pos_f = consts.tile([CH, L], f32)
            IW = max(bands[s][0][1] for s in range(BPC))
            IW2 = max(bands[s][1][1] for s in range(BPC))
            for lo, hi in ((0, IW), (IW, IW2), (IW2, L)):
                nc.gpsimd.iota(pos_f[:, lo:hi], pattern=[[1, hi - lo]], base=lo,
                               channel_multiplier=0,
                               allow_small_or_imprecise_dtypes=True)
            ym_sb = consts.tile([CH, BPC * LCN], f32)
            # W carries DerivErf's 2/sqrt(pi) factor; scaling eps by the same
            # factor makes rd = ym/(k*sumW + k*eps) = ym/k/(sumW + eps) exact.
            eps_sb = consts.tile([CH, 1], f32)
            nc.vector.memset(eps_sb, float(EPS) * 2.0 / np.pi ** 0.5)
            # warm the ACT spline tables during the input DMAs
            tblw = consts.tile([CH, 1], f32)
            nc.scalar.activation(out=tblw, in_=eps_sb, func=FT.Derivative_Erf)

            def col(tile_, idx):
                return tile_[:, idx:idx + 1]

            def cidx(q, b, t):
                return (q * BPC + b) * TCN + t

            xta_tiles = {}
            w_tiles = {}

            def load_xta(b):
                xta_sb = xta_p.tile([CH, TCN, C + 1], bf16)
                nc.sync.dma_start(out=xta_sb, in_=xta_d[b])
                if not trivial_masks:
                    for t in range(TCN):
                        # x_mask fold on GpSimd (broadcast mult, x cols only)
                        nc.gpsimd.tensor_tensor(
                            out=xta_sb[:, t, :C], in0=xta_sb[:, t, :C],
                            in1=bcast(col(coefs_sb, cidx(2, b, t)), C),
                            op=OP.mult,
                        )
                xta_tiles[b] = xta_sb

            def wgen(b, t):
                bs, be = bands[b][t]
                bw = be - bs
                # mu on DVE only: concurrent GpSimd streaming steals the DVE's
                # second SBUF port and drops tensor_scalar from 2x to 1x.
                mu = mu_p.tile([CH, wmax], f32, tag="mu")
                nc.vector.tensor_scalar(
                    out=mu[:, :bw], in0=pos_f[:, bs:be],
                    scalar1=col(coefs_sb, cidx(0, b, t)), scalar2=None,
                    op0=OP.subtract,
                )
                wt = w_pools[t].tile([CH, wmax], bf16)
                # W = 2/sqrt(pi) * exp(-(s*mu)^2); constant cancels via rd
                nc.scalar.activation(
                    out=wt[:, :bw], in_=mu[:, :bw], func=FT.Derivative_Erf,
                    scale=col(coefs_sb, cidx(1, b, t)),
                )
                w_tiles[(b, t)] = wt

            ogrp_live = {}

            def group(b, g):
                sb = bands[b]
                pgrp = psum_p.tile([CH, GRP, 512], f32, tag="pgrp")
                for k in range(GRP):
                    j = g * GRP + k
                    lo = j * CH
                    ctc = [t for t in range(TCN) if sb[t][0] <= lo and lo + CH <= sb[t][1]]
                    if not ctc:
                        nc.vector.memset(pgrp[:, k, :C + 1], 0.0)
                        continue
                    for i, t in enumerate(ctc):
                        off = lo - sb[t][0]
                        nc.tensor.matmul(
                            out=pgrp[:, k, :C + 1],
                            lhsT=w_tiles[(b, t)][:, off:off + CH],
                            rhs=xta_tiles[b][:, t, :],
                            start=(i == 0), stop=(i == len(ctc) - 1),
                        )
                dtmp = small_p.tile([CH, GRP], f32, tag="dtmp")
                # d + eps on DVE (same-engine chain: no cross-engine hop)
                nc.vector.tensor_scalar(
                    out=dtmp, in0=pgrp[:, :, C], scalar1=float(EPS) * 1.1283791670955126,
                    scalar2=None, op0=OP.add,
                )
                rd = small_p.tile([CH, GRP], f32, tag="rd")
                nc.vector.reciprocal(out=rd, in_=dtmp)
                if not trivial_masks:
                    nc.gpsimd.tensor_tensor(
                        out=rd, in0=rd,
                        in1=ym_sb[:, b * LCN + g * GRP: b * LCN + g * GRP + GRP],
                        op=OP.mult,
                    )
                # ogrp spans a PAIR of psum groups -> one out-DMA per pair
                half = g % 2
                if half == 0:
                    ogrp_new = out_p.tile([CH, 2 * GRP, C], f32, tag="ogrp")
                    ogrp_live[b] = ogrp_new
                ogrp = ogrp_live[b]
                osl = ogrp[:, half * GRP:(half + 1) * GRP, :]
                if (b * NGRP + g) % 4 < 3:
                    # normalize all chunks in one DVE op (rd broadcast on a
                    # stride-0 free dim)
                    rdb = bass.AP(tensor=rd.tensor, offset=rd.offset,
                                  ap=[rd.ap[0], rd.ap[1], [0, C]])
                    nc.vector.tensor_tensor(
                        out=osl, in0=pgrp[:, :, :C], in1=rdb, op=OP.mult,
                    )
                else:
                    for k in range(GRP):
                        nc.scalar.activation(
                            out=osl[:, k, :], in_=pgrp[:, k, :C],
                            func=FT.Copy, scale=col(rd, k),
                        )
                if half == 1:
                    pair = g // 2
                    eng = nc.sync if pair % 2 == 0 else nc.gpsimd
                    eng.dma_start(
                        out=out_d[b, pair * 2 * GRP * CH:(pair + 1) * 2 * GRP * CH, :]
                        .rearrange("(k p) c -> p k c", p=CH),
                        in_=ogrp,
                    )

            # batch 0 weight phase, then interleave batch 1's weight phase
            # into batch 0's matmul/normalize groups to keep all engines fed.
            # Emission order matters: each wgen's DMA-lane wait covers every
            # DMA emitted before it, so DMAs are interleaved to need-time.
            wgen(0, 0)          # needs iota1 + coefs only
            load_xta(0)
            wgen(0, 1)
            # PE warm-up on real data: back-to-back matmuls bridge the gap
            # until the group stream starts, so HAM un-throttles and the
            # real matmuls run at 2.4GHz.
            wps = psum_p.tile([CH, GRP, 512], f32, tag="pgrp")
            for i in range(12):
                nc.tensor.matmul(
                    out=wps[:, 0, :C + 1], lhsT=w_tiles[(0, 0)][:, :CH],
                    rhs=xta_tiles[0][:, 0, :], start=True, stop=True,
                )
            if not trivial_masks:
                nc.sync.dma_start(out=ym_sb, in_=ym_d[:, :])
            for t in range(2, TCN):
                wgen(0, t)
            load_xta(1)
            for g in range(NGRP):
                group(0, g)
                if g < 4:
                    wgen(1, g)
            for g in range(NGRP):
                group(1, g)
    return nc


def _prepare_inputs(x, w, x_mask, y_mask, sigma_scale):
    center, s = _center_scale(w, sigma_scale[0])
    bands = _bands(center, w)

    xt = np.ascontiguousarray(x.transpose(0, 2, 1))          # (B, T, C)
    xta = np.concatenate([xt, np.ones((B, T, 1), np.float32)], axis=2)
    # device layout [b, p, tc, c] for a descriptor-light DMA
    xta = np.ascontiguousarray(
        xta.reshape(B, TCN, CH, C + 1).transpose(0, 2, 1, 3)).astype(_bf16)

    xm = np.broadcast_to(x_mask.reshape(B, T), (B, T)).astype(np.float32)
    ymf = np.broadcast_to(y_mask.reshape(B, L), (B, L)).astype(np.float32)
    trivial_masks = bool(np.all(xm == 1.0) and np.all(ymf == 1.0))

    in_maps = []
    for core in range(N_CORES):
        bsel = [core * BPC + s_ for s_ in range(BPC)]
        coefs = np.empty((3, BPC, TCN, CH), np.float32)
        for s_, bb in enumerate(bsel):
            coefs[0, s_] = center[bb].reshape(TCN, CH)
            coefs[1, s_] = s[bb].reshape(TCN, CH)
            coefs[2, s_] = xm[bb].reshape(TCN, CH)
        ym_c = np.stack([ymf[bb].reshape(LCN, CH) for bb in bsel])  # (BPC,LCN,CH)
        in_maps.append({
            "xta": xta[bsel],
            "coefs": np.ascontiguousarray(
                coefs.reshape(3 * BPC * TCN, CH).T),          # [CH, 24]
            "ym": np.ascontiguousarray(
                ym_c.reshape(BPC * LCN, CH).T),               # [CH, 64]
        })
    band_key = (tuple(tuple(tuple(p) for p in sb) for sb in bands),
                trivial_masks)
    return in_maps, band_key


def kernel(x, w, x_mask, y_mask, sigma_scale):
    x = np.asarray(x, dtype=np.float32)
    w = np.asarray(w, dtype=np.float32)
    x_mask = np.asarray(x_mask, dtype=np.float32)
    y_mask = np.asarray(y_mask, dtype=np.float32)
    sigma_scale = np.asarray(sigma_scale, dtype=np.float32)
    assert x.shape == (B, C, T) and w.shape == (B, T)

    in_maps, band_key = _prepare_inputs(x, w, x_mask, y_mask, sigma_scale)

    if band_key not in _cache:
        nc = _build(band_key)
        _split_excess_waits(nc)
        _cache[band_key] = nc
    nc = _cache[band_key]

    from concourse.bass_utils import run_bass_kernel_spmd

    res = run_bass_kernel_spmd(nc, in_maps, list(range(N_CORES)), trace=False)
    outs = [res.results[i]["out"] for i in range(N_CORES)]      # (BPC, L, C) each
    full = np.concatenate(outs, axis=0)                          # (B, L, C)
    return full.transpose(0, 2, 1)                               # (B, C, L)


# revision 57
# speedup vs baseline: 1.0584x; 1.0119x over previous
"""DifferentiableLengthRegulator Trainium2 kernel.

out[b,c,l] = y_mask * (sum_t x[b,c,t]*W[b,t,l]) / (sum_t W[b,t,l] + eps)
W = exp(-0.5*(l - center[b,t])^2 / (w[b,t]^2*sigma_scale^2 + eps))

Sharding: data-parallel over batch B=16 -> 8 cores x 2 batches.
Per core, per batch (banded over the frame axis, since the Gaussian
weights vanish outside ~13 sigma of each token chunk's centers):
  DVE : mu = pos - c                        (tensor_scalar, 2x fp32)
  ACT : W  = DerivErf(s*mu) -> bf16         (= 2/sqrt(pi) * exp(-(s*mu)^2);
        the 2/sqrt(pi) factor cancels in the normalization)
  PE  : psum[l,0:257] = sum_tc W_tc[:,l-slice]^T @ [xT | ones]  (bf16)
  DVE/ACT/POOL: rd = y_mask/(psum[:,256]+eps);
        out_sb[l,c] = psum[l,0:256]*rd (PSUM->SBUF move, engine-balanced)
Output written (B, L, C)-contiguous; host returns the transpose view.
"""

import numpy as np
import ml_dtypes

B, C, T, L = 16, 256, 512, 4096
N_CORES = 8
BPC = B // N_CORES  # batches per core
CH = 128            # partition chunk
TCN = T // CH       # 4 token chunks
LCN = L // CH       # 32 frame chunks
GRP = 2             # frame chunks per psum group
NGRP = LCN // GRP   # 16 groups
EPS = 1e-8
MARGIN_SIGMA = 13.19
BAND_ALIGN = 128

_bf16 = ml_dtypes.bfloat16
_cache = {}


def _center_scale(w, sigma_scale):
    """Mirror the reference's cumsum/center math (same jax backend bits)."""
    try:
        import jax.numpy as jnp

        wj = jnp.asarray(w)
        center = np.asarray(jnp.cumsum(wj, axis=1) - 0.5 * wj, dtype=np.float32)
    except Exception:
        center = (np.cumsum(w, axis=1, dtype=np.float32) - 0.5 * w).astype(np.float32)
    sigma = (w * np.float32(sigma_scale)).astype(np.float32)
    # W = DerivErf(s*mu)*sqrt(pi)/2 = exp(-(s*mu)^2), s = sqrt(0.5/(sig^2+eps))
    s = np.sqrt(np.float32(0.5) / (np.square(sigma) + np.float32(EPS))).astype(np.float32)
    return center, s


def _bands(center, w_all):
    """Per (slot, tc) aligned frame band, unioned across cores (SPMD)."""
    margin = float(MARGIN_SIGMA * w_all.max() + 1.0)
    bands = []
    for slot in range(BPC):
        rows = center[slot::BPC]  # the 8 batches that land in this slot
        sb = []
        for tc in range(TCN):
            seg = rows[:, tc * CH:(tc + 1) * CH]
            bs = max(0, int(np.floor((seg.min() - margin) / BAND_ALIGN)) * BAND_ALIGN)
            be = min(L, int(np.ceil((seg.max() + margin) / BAND_ALIGN)) * BAND_ALIGN)
            if tc == 0:
                bs = 0
            if tc == TCN - 1:
                be = L
            bs = min(bs, be - CH)
            sb.append((bs, be))
        bands.append(sb)
    return bands


def _split_excess_waits(nc, max_waits=1):
    """walrus here caps sync-waits at 1 per compute instruction; move the
    excess onto injected same-engine NoOps just before the instruction
    (waiting earlier on the same engine is always safe)."""
    from concourse import mybir

    for f in nc.m.functions:
        for blk in f.blocks:
            new = []
            for inst in blk.instructions:
                si = inst.sync_info
                if si is not None and len(si.on_wait) > max_waits:
                    waits = list(si.on_wait)
                    keep, extra = waits[-max_waits:], waits[:-max_waits]
                    for i in range(0, len(extra), max_waits):
                        nop = mybir.InstNoOp(name=f"{inst.name}-xw{i}", ins=[], outs=[])
                        nop.engine = inst.engine
                        nop.sync_info = mybir.SyncInfo(
                            on_wait=extra[i:i + max_waits], on_update=[])
                        new.append(nop)
                    inst.sync_info = mybir.SyncInfo(
                        on_wait=keep, on_update=list(si.on_update))
                new.append(inst)
            blk.instructions = new


def _build(band_key):
    import concourse.bass as bass
    import concourse.tile as tile
    from concourse import mybir

    band_key, trivial_masks = band_key
    bands = [[(band_key[s][t][0], band_key[s][t][1]) for t in range(TCN)]
             for s in range(BPC)]
    wmax = max(be - bs for sb in bands for (bs, be) in sb)

    nc = bass.Bass("TRN2", target_bir_lowering=False, debug=False)
    # xta host layout: [b, p, tc, c] so the DMA is descriptor-light
    xta_d = nc.declare_dram_parameter("xta", [BPC, CH, TCN, C + 1], mybir.dt.bfloat16, isOutput=False)
    coefs_d = nc.declare_dram_parameter("coefs", [CH, 3 * BPC * TCN], mybir.dt.float32, isOutput=False)
    ym_d = nc.declare_dram_parameter("ym", [CH, BPC * LCN], mybir.dt.float32, isOutput=False)
    out_d = nc.declare_dram_parameter("out", [BPC, L, C], mybir.dt.float32, isOutput=True)

    f32 = mybir.dt.float32
    bf16 = mybir.dt.bfloat16
    FT = mybir.ActivationFunctionType
    OP = mybir.AluOpType

    def bcast(ap_col, n):
        return bass.AP(tensor=ap_col.tensor, offset=ap_col.offset,
                       ap=list(ap_col.ap) + [[0, n]])

    with tile.TileContext(nc) as tc_:
        import contextlib

        with contextlib.ExitStack() as ctx:
            consts = ctx.enter_context(tc_.tile_pool(name="consts", bufs=1))
            xta_p = ctx.enter_context(tc_.tile_pool(name="xta", bufs=2))
            mu_p = ctx.enter_context(tc_.tile_pool(name="mu", bufs=3))
            w_pools = [ctx.enter_context(tc_.tile_pool(name=f"w{t}", bufs=2)) for t in range(TCN)]
            psum_p = ctx.enter_context(tc_.tile_pool(name="ps", bufs=4, space="PSUM"))
            small_p = ctx.enter_context(tc_.tile_pool(name="small", bufs=6))
            out_p = ctx.enter_context(tc_.tile_pool(name="osb", bufs=4))

            # --- constants. pos comes entirely from iota on GpSimd (no DMA
            # wait at all); coefs on the sync HWDGE ring (keep ScalarE free
            # for the ACT table load -- dma_start blocks its issuing engine).
            coefs_sb = consts.tile([CH, 3 * BPC * TCN], f32)
            nc.sync.dma_start(out=coefs_sb, in_=coefs_d[:, :])
            pos_f = consts.tile([CH, L], f32)
            IW = max(bands[s][0][1] for s in range(BPC))
            IW2 = max(bands[s][1][1] for s in range(BPC))
            for lo, hi in ((0, IW), (IW, IW2), (IW2, L)):
                nc.gpsimd.iota(pos_f[:, lo:hi], pattern=[[1, hi - lo]], base=lo,
                               channel_multiplier=0,
                               allow_small_or_imprecise_dtypes=True)
            ym_sb = consts.tile([CH, BPC * LCN], f32)
            # W carries DerivErf's 2/sqrt(pi) factor; scaling eps by the same
            # factor makes rd = ym/(k*sumW + k*eps) = ym/k/(sumW + eps) exact.
            eps_sb = consts.tile([CH, 1], f32)
            nc.vector.memset(eps_sb, float(EPS) * 2.0 / np.pi ** 0.5)
            # warm the ACT spline tables during the input DMAs
            tblw = consts.tile([CH, 1], f32)
            nc.scalar.activation(out=tblw, in_=eps_sb, func=FT.Derivative_Erf)

            def col(tile_, idx):
                return tile_[:, idx:idx + 1]

            def cidx(q, b, t):
                return (q * BPC + b) * TCN + t

            xta_tiles = {}
            w_tiles = {}

            def load_xta(b):
                xta_sb = xta_p.tile([CH, TCN, C + 1], bf16)
                nc.sync.dma_start(out=xta_sb, in_=xta_d[b])
                if not trivial_masks:
                    for t in range(TCN):
                        # x_mask fold on GpSimd (broadcast mult, x cols only)
                        nc.gpsimd.tensor_tensor(
                            out=xta_sb[:, t, :C], in0=xta_sb[:, t, :C],
                            in1=bcast(col(coefs_sb, cidx(2, b, t)), C),
                            op=OP.mult,
                        )
                xta_tiles[b] = xta_sb

            def wgen(b, t):
                bs, be = bands[b][t]
                bw = be - bs
                # mu on DVE only: concurrent GpSimd streaming steals the DVE's
                # second SBUF port and drops tensor_scalar from 2x to 1x.
                mu = mu_p.tile([CH, wmax], f32, tag="mu")
                nc.vector.tensor_scalar(
                    out=mu[:, :bw], in0=pos_f[:, bs:be],
                    scalar1=col(coefs_sb, cidx(0, b, t)), scalar2=None,
                    op0=OP.subtract,
                )
                wt = w_pools[t].tile([CH, wmax], bf16)
                # W = 2/sqrt(pi) * exp(-(s*mu)^2); constant cancels via rd
                nc.scalar.activation(
                    out=wt[:, :bw], in_=mu[:, :bw], func=FT.Derivative_Erf,
                    scale=col(coefs_sb, cidx(1, b, t)),
                )
                w_tiles[(b, t)] = wt

            ogrp_live = {}

            def group(b, g):
                sb = bands[b]
                pgrp = psum_p.tile([CH, GRP, 512], f32, tag="pgrp")
                for k in range(GRP):
                    j = g * GRP + k
                    lo = j * CH
                    ctc = [t for t in range(TCN) if sb[t][0] <= lo and lo + CH <= sb[t][1]]
                    if not ctc:
                        nc.vector.memset(pgrp[:, k, :C + 1], 0.0)
                        continue
                    for i, t in enumerate(ctc):
                        off = lo - sb[t][0]
                        nc.tensor.matmul(
                            out=pgrp[:, k, :C + 1],
                            lhsT=w_tiles[(b, t)][:, off:off + CH],
                            rhs=xta_tiles[b][:, t, :],
                            start=(i == 0), stop=(i == len(ctc) - 1),
                        )
                dtmp = small_p.tile([CH, GRP], f32, tag="dtmp")
                # d + eps on DVE (same-engine chain: no cross-engine hop)
                nc.vector.tensor_scalar(
                    out=dtmp, in0=pgrp[:, :, C], scalar1=float(EPS) * 1.1283791670955126,
                    scalar2=None, op0=OP.add,
                )
                rd = small_p.tile([CH, GRP], f32, tag="rd")
                nc.vector.reciprocal(out=rd, in_=dtmp)
                if not trivial_masks:
                    nc.gpsimd.tensor_tensor(
                        out=rd, in0=rd,
                        in1=ym_sb[:, b * LCN + g * GRP: b * LCN + g * GRP + GRP],
                        op=OP.mult,
                    )
                # ogrp spans a PAIR of psum groups -> one out-DMA per pair
                half = g % 2
                if half == 0:
                    ogrp_new = out_p.tile([CH, 2 * GRP, C], f32, tag="ogrp")
                    ogrp_live[b] = ogrp_new
                ogrp = ogrp_live[b]
                osl = ogrp[:, half * GRP:(half + 1) * GRP, :]
                if (b * NGRP + g) % 4 < 3:
                    # normalize all chunks in one DVE op (rd broadcast on a
                    # stride-0 free dim)
                    rdb = bass.AP(tensor=rd.tensor, offset=rd.offset,
                                  ap=[rd.ap[0], rd.ap[1], [0, C]])
                    nc.vector.tensor_tensor(
                        out=osl, in0=pgrp[:, :, :C], in1=rdb, op=OP.mult,
                    )
                else:
                    for k in range(GRP):
                        nc.scalar.activation(
                            out=osl[:, k, :], in_=pgrp[:, k, :C],
                            func=FT.Copy, scale=col(rd, k),
                        )
                if half == 1:
                    pair = g // 2
                    eng = nc.sync if pair % 2 == 0 else nc.gpsimd
                    eng.dma_start(
                        out=out_d[b, pair * 2 * GRP * CH:(pair + 1) * 2 * GRP * CH, :]
                        .rearrange("(k p) c -> p k c", p=CH),
                        in_=ogrp,
                    )

            # batch 0 weight phase, then interleave batch 1's weight phase
            # into batch 0's matmul/normalize groups to keep all engines fed.
            # Emission order matters: each wgen's DMA-lane wait covers every
            # DMA emitted before it, so DMAs are interleaved to need-time.
            wgen(0, 0)          # needs iota1 + coefs only
            load_xta(0)
            wgen(0, 1)
            # PE warm-up on real data: back-to-back matmuls bridge the gap
            # until the group stream starts, so HAM un-throttles and the
            # real matmuls run at 2.4GHz.
            wps = psum_p.tile([CH, GRP, 512], f32, tag="pgrp")
            for i in range(12):
                nc.tensor.matmul(
                    out=wps[:, 0, :C + 1], lhsT=w_tiles[(0, 0)][:, :CH],
                    rhs=xta_tiles[0][:, 0, :], start=True, stop=True,
                )
            if not trivial_masks:
                nc.sync.dma_start(out=ym_sb, in_=ym_d[:, :])
            for t in range(2, TCN):
                wgen(0, t)
            load_xta(1)
            for g in range(NGRP // 2):
                group(0, g)
                if g in (1, 3, 5, 7):
                    wgen(1, g // 2)
            # interleave the second half of batch 0 with the start of batch 1
            # to flatten the transition and spread the normalize load
            for g in range(NGRP // 2, NGRP):
                group(0, g)
                group(1, g - NGRP // 2)
            for g in range(NGRP // 2, NGRP):
                group(1, g)
    return nc


def _prepare_inputs(x, w, x_mask, y_mask, sigma_scale):
    center, s = _center_scale(w, sigma_scale[0])
    bands = _bands(center, w)

    xt = np.ascontiguousarray(x.transpose(0, 2, 1))          # (B, T, C)
    xta = np.concatenate([xt, np.ones((B, T, 1), np.float32)], axis=2)
    # device layout [b, p, tc, c] for a descriptor-light DMA
    xta = np.ascontiguousarray(
        xta.reshape(B, TCN, CH, C + 1).transpose(0, 2, 1, 3)).astype(_bf16)

    xm = np.broadcast_to(x_mask.reshape(B, T), (B, T)).astype(np.float32)
    ymf = np.broadcast_to(y_mask.reshape(B, L), (B, L)).astype(np.float32)
    trivial_masks = bool(np.all(xm == 1.0) and np.all(ymf == 1.0))

    in_maps = []
    for core in range(N_CORES):
        bsel = [core * BPC + s_ for s_ in range(BPC)]
        coefs = np.empty((3, BPC, TCN, CH), np.float32)
        for s_, bb in enumerate(bsel):
            coefs[0, s_] = center[bb].reshape(TCN, CH)
            coefs[1, s_] = s[bb].reshape(TCN, CH)
            coefs[2, s_] = xm[bb].reshape(TCN, CH)
        ym_c = np.stack([ymf[bb].reshape(LCN, CH) for bb in bsel])  # (BPC,LCN,CH)
        in_maps.append({
            "xta": xta[bsel],
            "coefs": np.ascontiguousarray(
                coefs.reshape(3 * BPC * TCN, CH).T),          # [CH, 24]
            "ym": np.ascontiguousarray(
                ym_c.reshape(BPC * LCN, CH).T),               # [CH, 64]
        })
    band_key = (tuple(tuple(tuple(p) for p in sb) for sb in bands),
                trivial_masks)
    return in_maps, band_key


def kernel(x, w, x_mask, y_mask, sigma_scale):
    x = np.asarray(x, dtype=np.float32)
    w = np.asarray(w, dtype=np.float32)
    x_mask = np.asarray(x_mask, dtype=np.float32)
    y_mask = np.asarray(y_mask, dtype=np.float32)
    sigma_scale = np.asarray(sigma_scale, dtype=np.float32)
    assert x.shape == (B, C, T) and w.shape == (B, T)

    in_maps, band_key = _prepare_inputs(x, w, x_mask, y_mask, sigma_scale)

    if band_key not in _cache:
        nc = _build(band_key)
        _split_excess_waits(nc)
        _cache[band_key] = nc
    nc = _cache[band_key]

    from concourse.bass_utils import run_bass_kernel_spmd

    res = run_bass_kernel_spmd(nc, in_maps, list(range(N_CORES)), trace=False)
    outs = [res.results[i]["out"] for i in range(N_CORES)]      # (BPC, L, C) each
    full = np.concatenate(outs, axis=0)                          # (B, L, C)
    return full.transpose(0, 2, 1)                               # (B, C, L)


# revision 60
# speedup vs baseline: 1.0791x; 1.0195x over previous
"""DifferentiableLengthRegulator Trainium2 kernel.

out[b,c,l] = y_mask * (sum_t x[b,c,t]*W[b,t,l]) / (sum_t W[b,t,l] + eps)
W = exp(-0.5*(l - center[b,t])^2 / (w[b,t]^2*sigma_scale^2 + eps))

Sharding: data-parallel over batch B=16 -> 8 cores x 2 batches.
Per core, per batch (banded over the frame axis, since the Gaussian
weights vanish outside ~13 sigma of each token chunk's centers):
  DVE : mu = pos - c                        (tensor_scalar, 2x fp32)
  ACT : W  = DerivErf(s*mu) -> bf16         (= 2/sqrt(pi) * exp(-(s*mu)^2);
        the 2/sqrt(pi) factor cancels in the normalization)
  PE  : psum[l,0:257] = sum_tc W_tc[:,l-slice]^T @ [xT | ones]  (bf16)
  DVE/ACT/POOL: rd = y_mask/(psum[:,256]+eps);
        out_sb[l,c] = psum[l,0:256]*rd (PSUM->SBUF move, engine-balanced)
Output written (B, L, C)-contiguous; host returns the transpose view.
"""

import numpy as np
import ml_dtypes

B, C, T, L = 16, 256, 512, 4096
N_CORES = 8
BPC = B // N_CORES  # batches per core
CH = 128            # partition chunk
TCN = T // CH       # 4 token chunks
LCN = L // CH       # 32 frame chunks
GRP = 2             # frame chunks per psum group
NGRP = LCN // GRP   # 16 groups
EPS = 1e-8
MARGIN_SIGMA = 13.19
BAND_ALIGN = 128

_bf16 = ml_dtypes.bfloat16
_cache = {}


def _center_scale(w, sigma_scale):
    """Mirror the reference's cumsum/center math (same jax backend bits)."""
    try:
        import jax.numpy as jnp

        wj = jnp.asarray(w)
        center = np.asarray(jnp.cumsum(wj, axis=1) - 0.5 * wj, dtype=np.float32)
    except Exception:
        center = (np.cumsum(w, axis=1, dtype=np.float32) - 0.5 * w).astype(np.float32)
    sigma = (w * np.float32(sigma_scale)).astype(np.float32)
    # W = DerivErf(s*mu)*sqrt(pi)/2 = exp(-(s*mu)^2), s = sqrt(0.5/(sig^2+eps))
    s = np.sqrt(np.float32(0.5) / (np.square(sigma) + np.float32(EPS))).astype(np.float32)
    return center, s


def _bands(center, w_all):
    """Per (slot, tc) aligned frame band, unioned across cores (SPMD)."""
    margin = float(MARGIN_SIGMA * w_all.max() + 1.0)
    bands = []
    for slot in range(BPC):
        rows = center[slot::BPC]  # the 8 batches that land in this slot
        sb = []
        for tc in range(TCN):
            seg = rows[:, tc * CH:(tc + 1) * CH]
            bs = max(0, int(np.floor((seg.min() - margin) / BAND_ALIGN)) * BAND_ALIGN)
            be = min(L, int(np.ceil((seg.max() + margin) / BAND_ALIGN)) * BAND_ALIGN)
            if tc == 0:
                bs = 0
            if tc == TCN - 1:
                be = L
            bs = min(bs, be - CH)
            sb.append((bs, be))
        bands.append(sb)
    return bands


def _split_excess_waits(nc, max_waits=1):
    """walrus here caps sync-waits at 1 per compute instruction; move the
    excess onto injected same-engine NoOps just before the instruction
    (waiting earlier on the same engine is always safe)."""
    from concourse import mybir

    for f in nc.m.functions:
        for blk in f.blocks:
            new = []
            for inst in blk.instructions:
                si = inst.sync_info
                if si is not None and len(si.on_wait) > max_waits:
                    waits = list(si.on_wait)
                    keep, extra = waits[-max_waits:], waits[:-max_waits]
                    for i in range(0, len(extra), max_waits):
                        nop = mybir.InstNoOp(name=f"{inst.name}-xw{i}", ins=[], outs=[])
                        nop.engine = inst.engine
                        nop.sync_info = mybir.SyncInfo(
                            on_wait=extra[i:i + max_waits], on_update=[])
                        new.append(nop)
                    inst.sync_info = mybir.SyncInfo(
                        on_wait=keep, on_update=list(si.on_update))
                new.append(inst)
            blk.instructions = new


def _build(band_key):
    import concourse.bass as bass
    import concourse.tile as tile
    from concourse import mybir

    band_key, trivial_masks = band_key
    bands = [[(band_key[s][t][0], band_key[s][t][1]) for t in range(TCN)]
             for s in range(BPC)]
    wmax = max(be - bs for sb in bands for (bs, be) in sb)

    nc = bass.Bass("TRN2", target_bir_lowering=False, debug=False)
    # xta host layout: [b, p, tc, c] so the DMA is descriptor-light
    xta_d = nc.declare_dram_parameter("xta", [BPC, CH, TCN, C + 1], mybir.dt.bfloat16, isOutput=False)
    coefs_d = nc.declare_dram_parameter("coefs", [CH, 3 * BPC * TCN], mybir.dt.float32, isOutput=False)
    ym_d = nc.declare_dram_parameter("ym", [CH, BPC * LCN], mybir.dt.float32, isOutput=False)
    out_d = nc.declare_dram_parameter("out", [BPC, L, C], mybir.dt.float32, isOutput=True)

    f32 = mybir.dt.float32
    bf16 = mybir.dt.bfloat16
    FT = mybir.ActivationFunctionType
    OP = mybir.AluOpType

    def bcast(ap_col, n):
        return bass.AP(tensor=ap_col.tensor, offset=ap_col.offset,
                       ap=list(ap_col.ap) + [[0, n]])

    with tile.TileContext(nc) as tc_:
        import contextlib

        with contextlib.ExitStack() as ctx:
            consts = ctx.enter_context(tc_.tile_pool(name="consts", bufs=1))
            xta_p = ctx.enter_context(tc_.tile_pool(name="xta", bufs=2))
            mu_p = ctx.enter_context(tc_.tile_pool(name="mu", bufs=3))
            w_pools = [ctx.enter_context(tc_.tile_pool(name=f"w{t}", bufs=2)) for t in range(TCN)]
            psum_p = ctx.enter_context(tc_.tile_pool(name="ps", bufs=4, space="PSUM"))
            small_p = ctx.enter_context(tc_.tile_pool(name="small", bufs=6))
            out_p = ctx.enter_context(tc_.tile_pool(name="osb", bufs=4))

            # --- constants. pos comes entirely from iota on GpSimd (no DMA
            # wait at all); coefs on the sync HWDGE ring (keep ScalarE free
            # for the ACT table load -- dma_start blocks its issuing engine).
            coefs_sb = consts.tile([CH, 3 * BPC * TCN], f32)
            nc.sync.dma_start(out=coefs_sb, in_=coefs_d[:, :])
            pos_f = consts.tile([CH, L], f32)
            IW = max(bands[s][0][1] for s in range(BPC))
            IW2 = max(bands[s][1][1] for s in range(BPC))
            for lo, hi in ((0, IW), (IW, IW2), (IW2, L)):
                nc.gpsimd.iota(pos_f[:, lo:hi], pattern=[[1, hi - lo]], base=lo,
                               channel_multiplier=0,
                               allow_small_or_imprecise_dtypes=True)
            ym_sb = consts.tile([CH, BPC * LCN], f32)
            # W carries DerivErf's 2/sqrt(pi) factor; scaling eps by the same
            # factor makes rd = ym/(k*sumW + k*eps) = ym/k/(sumW + eps) exact.
            eps_sb = consts.tile([CH, 1], f32)
            nc.vector.memset(eps_sb, float(EPS) * 2.0 / np.pi ** 0.5)
            # warm the ACT spline tables during the input DMAs
            tblw = consts.tile([CH, 1], f32)
            nc.scalar.activation(out=tblw, in_=eps_sb, func=FT.Derivative_Erf)

            def col(tile_, idx):
                return tile_[:, idx:idx + 1]

            def cidx(q, b, t):
                return (q * BPC + b) * TCN + t

            xta_tiles = {}
            w_tiles = {}

            def load_xta(b):
                xta_sb = xta_p.tile([CH, TCN, C + 1], bf16)
                nc.sync.dma_start(out=xta_sb, in_=xta_d[b])
                if not trivial_masks:
                    for t in range(TCN):
                        # x_mask fold on GpSimd (broadcast mult, x cols only)
                        nc.gpsimd.tensor_tensor(
                            out=xta_sb[:, t, :C], in0=xta_sb[:, t, :C],
                            in1=bcast(col(coefs_sb, cidx(2, b, t)), C),
                            op=OP.mult,
                        )
                xta_tiles[b] = xta_sb

            def wgen(b, t, halves=1):
                bs, be = bands[b][t]
                bw = be - bs
                # mu on DVE only: concurrent GpSimd streaming steals the DVE's
                # second SBUF port and drops tensor_scalar from 2x to 1x.
                wt = w_pools[t].tile([CH, wmax], bf16)
                step = (bw // halves + CH - 1) // CH * CH
                for lo in range(0, bw, step):
                    hi = min(bw, lo + step)
                    mu = mu_p.tile([CH, wmax], f32, tag="mu")
                    nc.vector.tensor_scalar(
                        out=mu[:, :hi - lo], in0=pos_f[:, bs + lo:bs + hi],
                        scalar1=col(coefs_sb, cidx(0, b, t)), scalar2=None,
                        op0=OP.subtract,
                    )
                    # W = 2/sqrt(pi)*exp(-(s*mu)^2); constant cancels via rd
                    nc.scalar.activation(
                        out=wt[:, lo:hi], in_=mu[:, :hi - lo],
                        func=FT.Derivative_Erf,
                        scale=col(coefs_sb, cidx(1, b, t)),
                    )
                w_tiles[(b, t)] = wt

            ogrp_live = {}

            def group(b, g):
                sb = bands[b]
                pgrp = psum_p.tile([CH, GRP, 512], f32, tag="pgrp")
                for k in range(GRP):
                    j = g * GRP + k
                    lo = j * CH
                    ctc = [t for t in range(TCN) if sb[t][0] <= lo and lo + CH <= sb[t][1]]
                    if not ctc:
                        nc.vector.memset(pgrp[:, k, :C + 1], 0.0)
                        continue
                    for i, t in enumerate(ctc):
                        off = lo - sb[t][0]
                        nc.tensor.matmul(
                            out=pgrp[:, k, :C + 1],
                            lhsT=w_tiles[(b, t)][:, off:off + CH],
                            rhs=xta_tiles[b][:, t, :],
                            start=(i == 0), stop=(i == len(ctc) - 1),
                        )
                dtmp = small_p.tile([CH, GRP], f32, tag="dtmp")
                # d + eps on DVE (same-engine chain: no cross-engine hop)
                nc.vector.tensor_scalar(
                    out=dtmp, in0=pgrp[:, :, C], scalar1=float(EPS) * 1.1283791670955126,
                    scalar2=None, op0=OP.add,
                )
                rd = small_p.tile([CH, GRP], f32, tag="rd")
                nc.vector.reciprocal(out=rd, in_=dtmp)
                if not trivial_masks:
                    nc.gpsimd.tensor_tensor(
                        out=rd, in0=rd,
                        in1=ym_sb[:, b * LCN + g * GRP: b * LCN + g * GRP + GRP],
                        op=OP.mult,
                    )
                # ogrp spans a PAIR of psum groups -> one out-DMA per pair
                half = g % 2
                if half == 0:
                    ogrp_new = out_p.tile([CH, 2 * GRP, C], f32, tag="ogrp")
                    ogrp_live[b] = ogrp_new
                ogrp = ogrp_live[b]
                osl = ogrp[:, half * GRP:(half + 1) * GRP, :]
                tail = (b == 1 and g >= NGRP - 4)
                if tail:
                    # drain phase: split each group across DVE+ACT in parallel
                    # to shorten the trailing chain latency
                    nc.vector.tensor_scalar_mul(
                        out=osl[:, 0, :], in0=pgrp[:, 0, :C], scalar1=col(rd, 0))
                    nc.scalar.activation(
                        out=osl[:, 1, :], in_=pgrp[:, 1, :C],
                        func=FT.Copy, scale=col(rd, 1))
                elif (b * NGRP + g) % 4 < 3:
                    # normalize all chunks in one DVE op (rd broadcast on a
                    # stride-0 free dim)
                    rdb = bass.AP(tensor=rd.tensor, offset=rd.offset,
                                  ap=[rd.ap[0], rd.ap[1], [0, C]])
                    nc.vector.tensor_tensor(
                        out=osl, in0=pgrp[:, :, :C], in1=rdb, op=OP.mult,
                    )
                else:
                    for k in range(GRP):
                        nc.scalar.activation(
                            out=osl[:, k, :], in_=pgrp[:, k, :C],
                            func=FT.Copy, scale=col(rd, k),
                        )
                if b == 1 and g >= NGRP - 2:
                    # final stretch: DMA each group on its own ring so the
                    # last transfer starts as early as possible
                    eng = nc.sync if g % 2 == 0 else nc.gpsimd
                    eng.dma_start(
                        out=out_d[b, g * GRP * CH:(g + 1) * GRP * CH, :]
                        .rearrange("(k p) c -> p k c", p=CH),
                        in_=osl,
                    )
                elif half == 1:
                    pair = g // 2
                    eng = nc.sync if pair % 2 == 0 else nc.gpsimd
                    eng.dma_start(
                        out=out_d[b, pair * 2 * GRP * CH:(pair + 1) * 2 * GRP * CH, :]
                        .rearrange("(k p) c -> p k c", p=CH),
                        in_=ogrp,
                    )

            # batch 0 weight phase, then interleave batch 1's weight phase
            # into batch 0's matmul/normalize groups to keep all engines fed.
            # Emission order matters: each wgen's DMA-lane wait covers every
            # DMA emitted before it, so DMAs are interleaved to need-time.
            wgen(0, 0, halves=2)   # needs iota1 + coefs only; first half
            load_xta(0)            # unblocks the warm-up matmuls early
            wgen(0, 1)
            # PE warm-up on real data: back-to-back matmuls bridge the gap
            # until the group stream starts, so HAM un-throttles and the
            # real matmuls run at 2.4GHz.
            wps = psum_p.tile([CH, GRP, 512], f32, tag="pgrp")
            for i in range(12):
                nc.tensor.matmul(
                    out=wps[:, 0, :C + 1], lhsT=w_tiles[(0, 0)][:, :CH],
                    rhs=xta_tiles[0][:, 0, :], start=True, stop=True,
                )
            if not trivial_masks:
                nc.sync.dma_start(out=ym_sb, in_=ym_d[:, :])
            for t in range(2, TCN):
                wgen(0, t)
            load_xta(1)
            for g in range(NGRP // 2):
                group(0, g)
                if g in (1, 3, 5, 7):
                    wgen(1, g // 2)
            # interleave the second half of batch 0 with the start of batch 1
            # to flatten the transition and spread the normalize load
            for g in range(NGRP // 2, NGRP):
                group(0, g)
                group(1, g - NGRP // 2)
            for g in range(NGRP // 2, NGRP):
                group(1, g)
    return nc


def _prepare_inputs(x, w, x_mask, y_mask, sigma_scale):
    center, s = _center_scale(w, sigma_scale[0])
    bands = _bands(center, w)

    xt = np.ascontiguousarray(x.transpose(0, 2, 1))          # (B, T, C)
    xta = np.concatenate([xt, np.ones((B, T, 1), np.float32)], axis=2)
    # device layout [b, p, tc, c] for a descriptor-light DMA
    xta = np.ascontiguousarray(
        xta.reshape(B, TCN, CH, C + 1).transpose(0, 2, 1, 3)).astype(_bf16)

    xm = np.broadcast_to(x_mask.reshape(B, T), (B, T)).astype(np.float32)
    ymf = np.broadcast_to(y_mask.reshape(B, L), (B, L)).astype(np.float32)
    trivial_masks = bool(np.all(xm == 1.0) and np.all(ymf == 1.0))

    in_maps = []
    for core in range(N_CORES):
        bsel = [core * BPC + s_ for s_ in range(BPC)]
        coefs = np.empty((3, BPC, TCN, CH), np.float32)
        for s_, bb in enumerate(bsel):
            coefs[0, s_] = center[bb].reshape(TCN, CH)
            coefs[1, s_] = s[bb].reshape(TCN, CH)
            coefs[2, s_] = xm[bb].reshape(TCN, CH)
        ym_c = np.stack([ymf[bb].reshape(LCN, CH) for bb in bsel])  # (BPC,LCN,CH)
        in_maps.append({
            "xta": xta[bsel],
            "coefs": np.ascontiguousarray(
                coefs.reshape(3 * BPC * TCN, CH).T),          # [CH, 24]
            "ym": np.ascontiguousarray(
                ym_c.reshape(BPC * LCN, CH).T),               # [CH, 64]
        })
    band_key = (tuple(tuple(tuple(p) for p in sb) for sb in bands),
                trivial_masks)
    return in_maps, band_key


def kernel(x, w, x_mask, y_mask, sigma_scale):
    x = np.asarray(x, dtype=np.float32)
    w = np.asarray(w, dtype=np.float32)
    x_mask = np.asarray(x_mask, dtype=np.float32)
    y_mask = np.asarray(y_mask, dtype=np.float32)
    sigma_scale = np.asarray(sigma_scale, dtype=np.float32)
    assert x.shape == (B, C, T) and w.shape == (B, T)

    in_maps, band_key = _prepare_inputs(x, w, x_mask, y_mask, sigma_scale)

    if band_key not in _cache:
        nc = _build(band_key)
        _split_excess_waits(nc)
        _cache[band_key] = nc
    nc = _cache[band_key]

    from concourse.bass_utils import run_bass_kernel_spmd

    res = run_bass_kernel_spmd(nc, in_maps, list(range(N_CORES)), trace=False)
    outs = [res.results[i]["out"] for i in range(N_CORES)]      # (BPC, L, C) each
    full = np.concatenate(outs, axis=0)                          # (B, L, C)
    return full.transpose(0, 2, 1)                               # (B, C, L)
